# revision 18
# baseline (speedup 1.0000x reference)
"""GQA attention forward, 8-way sharded on Trainium2 (Bass/Tile).

Sharding: 2-way data-parallel over batch x 4-way tensor-parallel over heads.
Core c handles batch b=c//4 and TP rank j=c%4: q heads [8j,8j+8), kv heads
{2j, 2j+1}. Output projection is done after an intra-group AllToAll that
re-shards from head-features to sequence rows, so each core emits final
output rows [512j, 512j+512) of its batch.

All matmuls run as float32r (TF32-like) at 1 cycle/row.
"""

import sys

sys.path.insert(0, "/opt/trn_rl_repo")

import numpy as np
import concourse.bass as bass
import concourse.bacc as bacc
import concourse.mybir as mybir
from concourse import tile
from concourse.bass_utils import run_bass_kernel_spmd

F32 = mybir.dt.float32
R = mybir.dt.float32r
AF = mybir.ActivationFunctionType

B, T, D = 2, 2048, 2048
NH, NKV, HD = 32, 8, 64
TP = 4                      # tensor-parallel group size
HQ = NH // (B * TP // B)    # q heads per core = 8
QF = HQ * HD                # q features per core = 512
KVF = 2 * HD                # kv features per core = 128
TQ = T // TP                # output row shard = 512
NEG = -1.0e9

_cache = {}


def _build():
    nc = bacc.Bacc("TRN2", target_bir_lowering=False, debug=False, num_devices=8)

    xT = nc.dram_tensor("xT", [D, T], F32, kind="ExternalInput")
    wqs = nc.dram_tensor("wqs", [D, QF], F32, kind="ExternalInput")
    bqs = nc.dram_tensor("bqs", [4, 128, 1], F32, kind="ExternalInput")
    wks = nc.dram_tensor("wks", [D, KVF], F32, kind="ExternalInput")
    bks = nc.dram_tensor("bks", [128, 1], F32, kind="ExternalInput")
    wvs = nc.dram_tensor("wvs", [D, KVF], F32, kind="ExternalInput")
    bvs = nc.dram_tensor("bvs", [128, 1], F32, kind="ExternalInput")
    wo = nc.dram_tensor("wo", [D, D], F32, kind="ExternalInput")
    bo_rep = nc.dram_tensor("bo_rep", [128, D], F32, kind="ExternalInput")
    eye = nc.dram_tensor("eye", [128, 128], F32, kind="ExternalInput")
    triu = nc.dram_tensor("triu", [128, 128], F32, kind="ExternalInput")
    comb = nc.dram_tensor("comb", [128, 256], F32, kind="ExternalInput")
    ones2 = nc.dram_tensor("ones2", [128, 2], F32, kind="ExternalInput")
    ones64 = nc.dram_tensor("ones64", [1, 192], F32, kind="ExternalInput")
    out = nc.dram_tensor("out", [TQ, D], F32, kind="ExternalOutput")

    KT = D // 128  # 16 contraction tiles

    with tile.TileContext(nc) as tc:
      with tc.tile_pool(name="dramp", bufs=1, space="DRAM") as dramp:
        # 8-rank AllToAll (4-rank groups unsupported): chunk 4*g+q goes to
        # core 4*g+q. Each core writes its data into both batch-groups'
        # chunks, with the wrong group's copy zeroed via the per-core
        # ones64a/ones64b masks; receivers sum the two candidate rows.
        a2a_in = dramp.tile([8, QF, TQ], F32, name="a2a_in", tag="a2a_in")
        a2a_out = dramp.tile([8, QF, TQ], F32, name="a2a_out", tag="a2a_out")

        with tc.tile_pool(name="pers", bufs=1) as pers:
            # persistent activations (transposed layouts, f32r)
            qT = [pers.tile([128, T], R, name=f"qT{i}", tag=f"qT{i}")
                  for i in range(4)]
            kT = pers.tile([128, T], R, name="kT", tag="kT")
            va = [pers.tile([128, 130], R, name=f"va{i}", tag=f"va{i}")
                  for i in range(16)]
            triu_t = pers.tile([128, 128], F32, name="triu_t", tag="triu_t")
            comb_t = pers.tile([128, 256], F32, name="comb_t", tag="comb_t")
            eye_t = pers.tile([128, 128], R, name="eye_t", tag="eye_t")
            on64_t = pers.tile([1, 64], R, name="on64_t", tag="on64_t")
            on64ab = [pers.tile([1, 64], R, name=f"on64m{i}", tag=f"on64m{i}")
                      for i in range(2)]
            bq_t = [pers.tile([128, 1], F32, name=f"bq{i}", tag=f"bq{i}")
                    for i in range(4)]
            bk_t = pers.tile([128, 1], F32, name="bk_t", tag="bk_t")
            bv_t = pers.tile([128, 1], F32, name="bv_t", tag="bv_t")

            nc.sync.dma_start(triu_t[:], triu[:])
            nc.sync.dma_start(comb_t[:], comb[:])
            nc.sync.dma_start(eye_t[:], eye[:].bitcast(R))
            nc.sync.dma_start(on64_t[:], ones64[:, 0:64].bitcast(R))
            nc.sync.dma_start(on64ab[0][:], ones64[:, 64:128].bitcast(R))
            nc.sync.dma_start(on64ab[1][:], ones64[:, 128:192].bitcast(R))
            for i in range(4):
                nc.sync.dma_start(bq_t[i][:], bqs[i])
            nc.sync.dma_start(bk_t[:], bks[:])
            nc.sync.dma_start(bv_t[:], bvs[:])

            # ---------------- phase 1: q/k/v projections ----------------
            with tc.tile_pool(name="wp", bufs=1) as wp, \
                 tc.tile_pool(name="xcp", bufs=2) as xcp, \
                 tc.tile_pool(name="vtp", bufs=2) as vtp, \
                 tc.tile_pool(name="ps1", bufs=2, space="PSUM") as ps1, \
                 tc.tile_pool(name="pst", bufs=2, space="PSUM") as pst:
                wq_t = {}
                for k in range(KT):
                    for ct in range(4):
                        t_ = wp.tile([128, 128], R, name=f"wq{k}_{ct}",
                                     tag=f"wq{k}_{ct}")
                        nc.sync.dma_start(
                            t_[:], wqs[128 * k:128 * k + 128,
                                       128 * ct:128 * ct + 128].bitcast(R))
                        wq_t[k, ct] = t_
                wk_t, wv_t = [], []
                for k in range(KT):
                    t_ = wp.tile([128, 128], R, name=f"wk{k}", tag=f"wk{k}")
                    nc.sync.dma_start(
                        t_[:], wks[128 * k:128 * k + 128, :].bitcast(R))
                    wk_t.append(t_)
                    t_ = wp.tile([128, 128], R, name=f"wv{k}", tag=f"wv{k}")
                    nc.sync.dma_start(
                        t_[:], wvs[128 * k:128 * k + 128, :].bitcast(R))
                    wv_t.append(t_)

                for tch in range(4):  # 512-wide t chunks
                    t0 = 512 * tch
                    xc = []
                    for k in range(KT):
                        t_ = xcp.tile([128, 512], R, name=f"xc{k}", tag=f"xc{k}")
                        nc.sync.dma_start(
                            t_[:], xT[128 * k:128 * k + 128,
                                      t0:t0 + 512].bitcast(R))
                        xc.append(t_)
                    for ct in range(4):  # q
                        ps = ps1.tile([128, 512], F32, name="ps_q", tag="psq")
                        for k in range(KT):
                            nc.tensor.matmul(ps[:], lhsT=wq_t[k, ct][:],
                                             rhs=xc[k][:], start=(k == 0),
                                             stop=(k == KT - 1))
                        nc.scalar.activation(qT[ct][:, t0:t0 + 512], ps[:],
                                             AF.Identity, bias=bq_t[ct][:])
                    ps = ps1.tile([128, 512], F32, name="ps_k", tag="psq")
                    for k in range(KT):
                        nc.tensor.matmul(ps[:], lhsT=wk_t[k][:], rhs=xc[k][:],
                                         start=(k == 0), stop=(k == KT - 1))
                    nc.scalar.activation(kT[:, t0:t0 + 512], ps[:],
                                         AF.Identity, bias=bk_t[:])
                    # v^T then transpose to natural [t, feat] with ones cols
                    ps = ps1.tile([128, 512], F32, name="ps_v", tag="psq")
                    for k in range(KT):
                        nc.tensor.matmul(ps[:], lhsT=wv_t[k][:], rhs=xc[k][:],
                                         start=(k == 0), stop=(k == KT - 1))
                    vt_sb = vtp.tile([128, 512], R, name="vt_sb", tag="vt")
                    nc.scalar.activation(vt_sb[:], ps[:], AF.Identity,
                                         bias=bv_t[:])
                    for st in range(4):
                        ti = 4 * tch + st
                        tp = pst.tile([128, 128], R, name="tp_v", tag="tpv")
                        nc.tensor.transpose(tp[:],
                                            vt_sb[:, 128 * st:128 * st + 128],
                                            eye_t[:])
                        nc.vector.tensor_copy(va[ti][:, 0:64], tp[:, 0:64])
                        nc.vector.tensor_copy(va[ti][:, 65:129], tp[:, 64:128])
                        nc.sync.dma_start(va[ti][:, 64:65],
                                          ones2[:, 0:1].bitcast(R))
                        nc.sync.dma_start(va[ti][:, 129:130],
                                          ones2[:, 1:2].bitcast(R))

            # ---------------- phase 2: attention ----------------
            with tc.tile_pool(name="scp", bufs=3, space="PSUM") as scp, \
                 tc.tile_pool(name="op", bufs=2, space="PSUM") as op, \
                 tc.tile_pool(name="rbp", bufs=2, space="PSUM") as rbp, \
                 tc.tile_pool(name="ep", bufs=4) as ep, \
                 tc.tile_pool(name="oup", bufs=2) as oup, \
                 tc.tile_pool(name="rrp", bufs=2) as rrp, \
                 tc.tile_pool(name="onp", bufs=3) as onp:
                for h in range(HQ):
                    # qT tile i holds kv-group-0 head i on partitions 0-63 and
                    # kv-group-1 head i on 64-127, so lhsT/rhs share a base.
                    g, i = h // 4, h % 4
                    hloc = 4 * g + i  # feature-block index in original order
                    qt_tile = qT[i]
                    qr = 64 * g
                    ou_h = oup.tile([64, T], F32, name="ou_h", tag="ou")
                    rr_h = rrp.tile([1, T], R, name="rr_h", tag="rr")
                    for tch in range(8):  # 256-wide chunks
                        t0 = 256 * tch
                        ns = 2 * tch + 2
                        ops = op.tile([65, 256], F32, name="ops", tag="ops")
                        for sb in range(ns):
                            s0 = 128 * sb
                            sc = scp.tile([128, 256], F32, name="sc", tag="sc")
                            nc.tensor.matmul(
                                sc[:],
                                lhsT=kT[64 * g:64 * g + 64, s0:s0 + 128],
                                rhs=qt_tile[qr:qr + 64, t0:t0 + 256],
                                start=True, stop=True)
                            if s0 == t0:
                                nc.vector.tensor_add(sc[:, 0:128],
                                                     sc[:, 0:128], triu_t[:])
                            elif s0 == t0 + 128:
                                nc.vector.tensor_add(sc[:], sc[:], comb_t[:])
                            e_t = ep.tile([128, 256], R, name="e_t", tag="e")
                            nc.scalar.activation(e_t[:], sc[:], AF.Exp)
                            nc.tensor.matmul(
                                ops[:],
                                lhsT=va[sb][:, 65 * g:65 * g + 65],
                                rhs=e_t[:], start=(sb == 0),
                                stop=(sb == ns - 1))
                        nc.vector.tensor_copy(ou_h[:, t0:t0 + 256],
                                              ops[0:64, :])
                        with nc.allow_low_precision(
                                reason="f32r softmax denom, 4B wide"):
                            nc.vector.reciprocal(rr_h[:, t0:t0 + 256],
                                                 ops[64:65, :])
                    # normalize + scatter to a2a_in
                    for nchunk in range(4):
                        n0 = 512 * nchunk
                        for gi in range(2):
                            rb = rbp.tile([64, 512], F32, name="rb", tag="rb")
                            nc.tensor.matmul(rb[:], lhsT=on64ab[gi][:],
                                             rhs=rr_h[0:1, n0:n0 + 512],
                                             start=True, stop=True)
                            on_t = onp.tile([64, 512], R, name="on_t",
                                            tag="on")
                            nc.vector.tensor_mul(on_t[:],
                                                 ou_h[:, n0:n0 + 512], rb[:])
                            nc.sync.dma_start(
                                a2a_in[4 * gi + nchunk,
                                       64 * hloc:64 * hloc + 64, :],
                                on_t[:].bitcast(F32))

            nc.gpsimd.collective_compute(
                "AllToAll", mybir.AluOpType.bypass,
                replica_groups=[[0, 1, 2, 3, 4, 5, 6, 7]],
                ins=[a2a_in.opt()], outs=[a2a_out.opt()])

        # ---------------- phase 3: output projection ----------------
        with tc.tile_pool(name="gthp", bufs=1) as gthp, \
             tc.tile_pool(name="wop", bufs=2) as wop, \
             tc.tile_pool(name="bop", bufs=1) as bop, \
             tc.tile_pool(name="outp", bufs=3) as outp, \
             tc.tile_pool(name="ps3", bufs=4, space="PSUM") as ps3:
            bo_t = bop.tile([128, D], F32, name="bo_t", tag="bo")
            nc.sync.dma_start(bo_t[:], bo_rep[:])
            gth = []
            for i in range(KT):
                ta = gthp.tile([128, TQ], F32, name=f"gta{i}", tag=f"gta{i}")
                nc.sync.dma_start(
                    ta[:], a2a_out[i // 4,
                                   128 * (i % 4):128 * (i % 4) + 128, :])
                tb = gthp.tile([128, TQ], F32, name=f"gtb{i}", tag=f"gtb{i}")
                nc.sync.dma_start(
                    tb[:], a2a_out[4 + i // 4,
                                   128 * (i % 4):128 * (i % 4) + 128, :])
                t_ = gthp.tile([128, TQ], R, name=f"gth{i}", tag=f"gth{i}")
                with nc.allow_low_precision(reason="f32r gather merge, 4B"):
                    nc.vector.tensor_add(t_[:], ta[:], tb[:])
                gth.append(t_)
            for n in range(4):
                n0 = 512 * n
                wo_n = []
                for k in range(KT):
                    t_ = wop.tile([128, 512], R, name=f"wo{k}", tag=f"wo{k}")
                    nc.sync.dma_start(
                        t_[:], wo[128 * k:128 * k + 128,
                                  n0:n0 + 512].bitcast(R))
                    wo_n.append(t_)
                for m in range(4):
                    ps = ps3.tile([128, 512], F32, name="ps_o", tag="pso")
                    for k in range(KT):
                        nc.tensor.matmul(ps[:],
                                         lhsT=gth[k][:, 128 * m:128 * m + 128],
                                         rhs=wo_n[k][:], start=(k == 0),
                                         stop=(k == KT - 1))
                    ot = outp.tile([128, 512], F32, name="ot", tag="ot")
                    nc.vector.tensor_add(ot[:], ps[:], bo_t[:, n0:n0 + 512])
                    nc.sync.dma_start(out[128 * m:128 * m + 128, n0:n0 + 512],
                                      ot[:])

    nc.compile()
    return nc


def kernel(x, mask, wq, bq, wk, bk, wv, bv, wo, bo, trace=False):
    if "nc" not in _cache:
        _cache["nc"] = _build()
    nc = _cache["nc"]

    x = np.asarray(x, np.float32)
    wq = np.asarray(wq, np.float32)
    bq = np.asarray(bq, np.float32)
    wk = np.asarray(wk, np.float32)
    bk = np.asarray(bk, np.float32)
    wv = np.asarray(wv, np.float32)
    bv = np.asarray(bv, np.float32)
    wo_f = np.ascontiguousarray(np.asarray(wo, np.float32))
    bo = np.asarray(bo, np.float32)

    xTb = [np.ascontiguousarray(x[b].T) for b in range(B)]
    bo_rep = np.ascontiguousarray(np.tile(bo[None, :], (128, 1)))
    eye = np.eye(128, dtype=np.float32)
    ii = np.arange(128)
    triu = np.where(ii[None, :] < ii[:, None], NEG, 0.0).astype(np.float32)
    comb = np.concatenate([np.full((128, 128), NEG, np.float32), triu], axis=1)
    comb = np.ascontiguousarray(comb)
    ones2 = np.ones((128, 2), np.float32)

    in_maps = []
    for c in range(8):
        b, j = c // 4, c % 4
        ks = slice(KVF * j, KVF * j + KVF)
        # qT tile i pairs local head i (kv group 0, partitions 0-63) with
        # local head 4+i (kv group 1, partitions 64-127)
        qcols = np.concatenate([
            np.arange(64 * (8 * j + i + 4 * g), 64 * (8 * j + i + 4 * g) + 64)
            for i in range(4) for g in range(2)])
        in_maps.append({
            "xT": xTb[b],
            "wqs": np.ascontiguousarray(wq[:, qcols] * 0.125),
            "bqs": np.ascontiguousarray(
                (bq[qcols] * 0.125).reshape(4, 128, 1)),
            "wks": np.ascontiguousarray(wk[:, ks]),
            "bks": np.ascontiguousarray(bk[ks].reshape(128, 1)),
            "wvs": np.ascontiguousarray(wv[:, ks]),
            "bvs": np.ascontiguousarray(bv[ks].reshape(128, 1)),
            "wo": wo_f,
            "bo_rep": bo_rep,
            "eye": eye,
            "triu": triu,
            "comb": comb,
            "ones2": ones2,
            "ones64": np.ascontiguousarray(np.concatenate([
                np.ones((1, 64), np.float32),
                np.full((1, 64), 1.0 if b == 0 else 0.0, np.float32),
                np.full((1, 64), 1.0 if b == 1 else 0.0, np.float32)],
                axis=1)),
        })

    try:
        res = run_bass_kernel_spmd(nc, in_maps, list(range(8)), trace=trace)
    except ModuleNotFoundError:
        trace = False
        res = run_bass_kernel_spmd(nc, in_maps, list(range(8)))
    full = np.empty((B, T, D), np.float32)
    for c in range(8):
        b, j = c // 4, c % 4
        full[b, TQ * j:TQ * j + TQ, :] = res.results[c]["out"]
    if trace:
        return full, res
    kernel.last_res = res
    return full
